# revision 10
# baseline (speedup 1.0000x reference)
"""CausalLocalSGU Trainium2 kernel.

Reference computation (per batch b):
  split x[b] channels -> res (first 1024), gate_in (last 1024)
  per 128-token window block j: z_j = LayerNorm(gate_in_j) * gamma + beta
  gate_out_j[m, c] = sum_n W[h(c), m, n] * [z_{j-1}; z_j][n, c] + bias[h(c), m]
      (W masked causally: keep [m, n] where n <= m + 128; z_{-1} = 0)
  out_j = gate_out_j * res_j

Sharding: 8 cores; core k handles batch k//2, token half k%2 (2048 tokens =
16 window blocks) plus a one-block halo on the left (zeros for even cores).
The LN of the halo block is recomputed locally -> no collectives.

Device pipeline per block (fast path: gamma==1, beta==0):
  LN stats via two ScalarE passes with accum_out (sum, sum of squares), tiny
  VectorE ops for mu/var, rstd via ACT sqrt + DVE reciprocal, normalize on
  ScalarE (Identity, scale=rstd, bias=-mu*rstd) into a bf16 z tile, 8 bf16
  matmuls (prev/current window per head; causal mask + transpose pre-applied
  on host), then one scalar_tensor_tensor per head on VectorE:
  out = (psum + bias[h]) * res, and DMA out.

DMA is batched 4 window blocks per transfer (1-2 MiB per dma_start).

The gate half is cast to bf16 on the host: the einsum term it feeds
contributes ~7e-5 of the output magnitude (weights ~1e-5), so bf16 there
perturbs the output by ~3e-7 relative while halving LN traffic and PE time.

General path (gamma != 1 or beta != 0) additionally multiplies z by gamma
(DVE) and folds beta+bias through an fp32 K=2 matmul:
  W @ (gamma*z + beta) = W @ (gamma*z) + S[m]*beta[c],  S[m] = sum_n W[m, n]
(S excludes the first window's columns when the previous block is zero pad).
"""

import ml_dtypes
import numpy as np

import concourse.bacc as bacc
import concourse.bass as bass
import concourse.tile as tile
from concourse import mybir
from concourse.bass_utils import run_bass_kernel_spmd

F32 = mybir.dt.float32
BF16 = mybir.dt.bfloat16

HEADS = 4
W = 128            # window
DIM = 2048
DOUT = 1024        # dim // 2
DHEAD = DOUT // HEADS  # 256
B = 4
N = 4096
NCORES = 8
BLK_PER_CORE = (N // 2) // W   # 16
MACRO = 4          # window blocks per DMA batch
LN_EPS = 1e-5

# fp32 consts layout (columns of the [128, 2056] consts tensor)
_BIAS0 = 0          # 4 cols: bias[h, :] as per-partition columns
_EXF0 = 8           # [2, 512] rows 0..1: bias / S for the first block
_EXR0 = 520         # [2, 512] rows 0..1: bias / S for the other blocks
_RHSX0 = 1032       # [2, 1024] row 0: ones, row 1: beta
_CONSTS_COLS = 2056

_NC_CACHE: dict = {}
_last_in_maps: list = []


def _build_nc(general: bool) -> bass.Bass:
    nc = bacc.Bacc(
        trn_type="TRN2",
        target_bir_lowering=False,
        debug=False,
        num_devices=NCORES,
    )
    nblk = BLK_PER_CORE  # output blocks per core; +1 halo block for gate
    res_sh = nc.dram_tensor("res_sh", [nblk * W, DOUT], F32, kind="ExternalInput").ap()
    gate_sh = nc.dram_tensor(
        "gate_sh", [(nblk + 1) * W, DOUT], BF16, kind="ExternalInput"
    ).ap()
    consts = nc.dram_tensor("consts", [W, _CONSTS_COLS], F32, kind="ExternalInput").ap()
    consts_bf = nc.dram_tensor(
        "consts_bf", [W, 2 * HEADS * W], BF16, kind="ExternalInput"
    ).ap()
    if general:
        gamma = nc.dram_tensor("gamma", [DOUT], F32, kind="ExternalInput").ap()
    out = nc.dram_tensor("out", [nblk * W, DOUT], F32, kind="ExternalOutput").ap()

    ident = mybir.ActivationFunctionType.Identity
    alu = mybir.AluOpType

    with tile.TileContext(nc) as tc:
        with (
            tc.tile_pool(name="singles", bufs=1) as singles,
            tc.tile_pool(name="gpool", bufs=3) as gpool,
            tc.tile_pool(name="rpool", bufs=3) as rpool,
            tc.tile_pool(name="opool", bufs=3) as opool,
            tc.tile_pool(name="zpool", bufs=4) as zpool,
            tc.tile_pool(name="spool", bufs=6) as spool,
            tc.tile_pool(name="scratch", bufs=2) as scratch,
            tc.tile_pool(name="ppool", bufs=4, space="PSUM") as ppool,
        ):
            consts_t = singles.tile([W, _CONSTS_COLS], F32)
            nc.sync.dma_start(out=consts_t, in_=consts)
            wt_t = singles.tile([W, 2 * HEADS * W], BF16)
            nc.sync.dma_start(out=wt_t, in_=consts_bf)
            exf_t = consts_t[0:2, _EXF0 : _EXF0 + HEADS * W]
            exr_t = consts_t[0:2, _EXR0 : _EXR0 + HEADS * W]
            rhsx_t = consts_t[0:2, _RHSX0 : _RHSX0 + DOUT]

            eps_t = singles.tile([128, 1], F32)
            nc.vector.memset(eps_t, LN_EPS)
            if general:
                gamma_t = singles.tile([128, DOUT], F32)
                nc.gpsimd.dma_start(
                    out=gamma_t,
                    in_=bass.AP(
                        tensor=gamma.tensor,
                        offset=gamma.offset,
                        ap=[[0, 128]] + list(gamma.ap),
                    ),
                )

            def layer_norm(gate):
                """gate: [W, DOUT] bf16 SBUF slice -> z (bf16 tile)."""
                zjunk = scratch.tile([W, DOUT], BF16, tag="zjunk")
                sums = spool.tile([W, 1], F32, tag="sums")
                nc.scalar.activation(
                    out=zjunk, in_=gate, func=ident, accum_out=sums
                )
                zjunk2 = scratch.tile([W, DOUT], BF16, tag="zjunk2")
                sumsq = spool.tile([W, 1], F32, tag="sumsq")
                nc.scalar.activation(
                    out=zjunk2,
                    in_=gate,
                    func=mybir.ActivationFunctionType.Square,
                    accum_out=sumsq,
                )
                mu = spool.tile([W, 1], F32, tag="mu")
                nc.vector.tensor_scalar_mul(mu, sums, 1.0 / DOUT)
                mu2 = spool.tile([W, 1], F32, tag="mu2")
                nc.vector.tensor_mul(mu2, mu, mu)
                var = spool.tile([W, 1], F32, tag="var")
                nc.vector.scalar_tensor_tensor(
                    out=var,
                    in0=sumsq,
                    scalar=1.0 / DOUT,
                    in1=mu2,
                    op0=alu.mult,
                    op1=alu.subtract,
                )
                sd = spool.tile([W, 1], F32, tag="sd")
                nc.scalar.activation(
                    out=sd,
                    in_=var,
                    func=mybir.ActivationFunctionType.Sqrt,
                    bias=eps_t,
                )
                rstd = spool.tile([W, 1], F32, tag="rstd")
                nc.vector.reciprocal(out=rstd, in_=sd)
                negmu = spool.tile([W, 1], F32, tag="negmu")
                nc.vector.tensor_scalar(
                    out=negmu,
                    in0=mu,
                    scalar1=rstd,
                    scalar2=-1.0,
                    op0=alu.mult,
                    op1=alu.mult,
                )
                z = zpool.tile([W, DOUT], BF16, tag="z")
                nc.scalar.activation(
                    out=z, in_=gate, func=ident, bias=negmu, scale=rstd
                )
                if general:
                    nc.vector.tensor_mul(z, z, gamma_t)
                return z

            # halo block
            gate0 = gpool.tile([W, DOUT], BF16, tag="gate0")
            nc.sync.dma_start(out=gate0, in_=gate_sh[0:W, :])
            z_prev = layer_norm(gate0)

            nmac = nblk // MACRO
            for m in range(nmac):
                g4 = gpool.tile([W, MACRO, DOUT], BF16, tag="g4")
                nc.sync.dma_start(
                    out=g4,
                    in_=gate_sh[(1 + m * MACRO) * W : (1 + (m + 1) * MACRO) * W, :]
                    .rearrange("(b p) d -> p b d", p=W),
                )
                r4 = rpool.tile([W, MACRO, DOUT], F32, tag="r4")
                nc.sync.dma_start(
                    out=r4,
                    in_=res_sh[m * MACRO * W : (m + 1) * MACRO * W, :]
                    .rearrange("(b p) d -> p b d", p=W),
                )
                o4 = opool.tile([W, MACRO, DOUT], F32, tag="o4")
                for s in range(MACRO):
                    blk = m * MACRO + s       # output block index 0..15
                    z = layer_norm(g4[:, s, :])
                    psum = ppool.tile([W, DOUT], F32, tag="psum")
                    ex_t = exf_t if blk == 0 else exr_t
                    for h in range(HEADS):
                        ps = psum[:, h * DHEAD : (h + 1) * DHEAD]
                        zp = z_prev[:, h * DHEAD : (h + 1) * DHEAD]
                        zc = z[:, h * DHEAD : (h + 1) * DHEAD]
                        if general:
                            nc.tensor.matmul(
                                ps,
                                ex_t[:, h * W : (h + 1) * W],
                                rhsx_t[:, h * DHEAD : (h + 1) * DHEAD],
                                start=True,
                                stop=False,
                            )
                        nc.tensor.matmul(
                            ps,
                            wt_t[:, (2 * h) * W : (2 * h + 1) * W],
                            zp,
                            start=not general,
                            stop=False,
                        )
                        nc.tensor.matmul(
                            ps,
                            wt_t[:, (2 * h + 1) * W : (2 * h + 2) * W],
                            zc,
                            start=False,
                            stop=True,
                        )
                    if general:
                        # bias/beta already folded in via the extras matmul
                        nc.vector.tensor_mul(o4[:, s, :], psum, r4[:, s, :])
                    else:
                        for h in range(HEADS):
                            hs = slice(h * DHEAD, (h + 1) * DHEAD)
                            nc.vector.scalar_tensor_tensor(
                                out=o4[:, s, hs],
                                in0=psum[:, hs],
                                scalar=consts_t[:, _BIAS0 + h : _BIAS0 + h + 1],
                                in1=r4[:, s, hs],
                                op0=alu.add,
                                op1=alu.mult,
                            )
                    z_prev = z
                nc.sync.dma_start(
                    out=out[m * MACRO * W : (m + 1) * MACRO * W, :]
                    .rearrange("(b p) d -> p b d", p=W),
                    in_=o4,
                )
    if not nc.is_finalized():
        nc.finalize()
    return nc


def _host_prep(weight, bias, ln_beta):
    j = np.arange(2 * W)[None, :]
    i_ = np.arange(W)[:, None]
    mask = (j <= i_ + W).astype(np.float32)          # [W, 2W]
    wm = weight * mask[None]                         # [H, W, 2W]
    wT = np.zeros((W, 2 * HEADS, W), dtype=np.float32)
    for h in range(HEADS):
        wT[:, 2 * h] = wm[h, :, :W].T                # A_h: prev-window cols
        wT[:, 2 * h + 1] = wm[h, :, W:].T            # B_h: current-window cols
    wT = wT.reshape(W, 2 * HEADS * W)

    s_full = wm.sum(-1).reshape(HEADS * W)
    s_first = wm[:, :, W:].sum(-1).reshape(HEADS * W)
    bias_flat = bias.reshape(HEADS * W)

    def consts_for(first_has_prev: bool):
        c = np.zeros((W, _CONSTS_COLS), dtype=np.float32)
        for h in range(HEADS):
            c[:, _BIAS0 + h] = bias[h]
        c[0, _EXF0 : _EXF0 + HEADS * W] = bias_flat
        c[1, _EXF0 : _EXF0 + HEADS * W] = s_full if first_has_prev else s_first
        c[0, _EXR0 : _EXR0 + HEADS * W] = bias_flat
        c[1, _EXR0 : _EXR0 + HEADS * W] = s_full
        c[0, _RHSX0 : _RHSX0 + DOUT] = 1.0
        c[1, _RHSX0 : _RHSX0 + DOUT] = ln_beta
        return c

    consts_bf = np.ascontiguousarray(wT.astype(ml_dtypes.bfloat16))
    return consts_for(False), consts_for(True), consts_bf


def kernel(x, weight, bias, ln_gamma, ln_beta):
    x = np.ascontiguousarray(x, dtype=np.float32)
    weight = np.asarray(weight, dtype=np.float32)
    bias = np.asarray(bias, dtype=np.float32)
    ln_gamma = np.asarray(ln_gamma, dtype=np.float32)
    ln_beta = np.asarray(ln_beta, dtype=np.float32)

    consts_even, consts_odd, consts_bf = _host_prep(weight, bias, ln_beta)

    general = not (np.all(ln_gamma == 1.0) and np.all(ln_beta == 0.0))
    if general not in _NC_CACHE:
        _NC_CACHE[general] = _build_nc(general)
    nc = _NC_CACHE[general]

    half = N // 2
    gate_bf = np.ascontiguousarray(x[:, :, DOUT:]).astype(ml_dtypes.bfloat16)
    in_maps = []
    for k in range(NCORES):
        bk, hk = k // 2, k % 2
        res_sh = np.ascontiguousarray(x[bk, hk * half : (hk + 1) * half, :DOUT])
        if hk == 0:
            halo = np.zeros((W, DOUT), dtype=ml_dtypes.bfloat16)
        else:
            halo = gate_bf[bk, half - W : half]
        gate_sh = np.ascontiguousarray(
            np.concatenate([halo, gate_bf[bk, hk * half : (hk + 1) * half]], axis=0)
        )
        m = {
            "res_sh": res_sh,
            "gate_sh": gate_sh,
            "consts": consts_odd if hk == 1 else consts_even,
            "consts_bf": consts_bf,
        }
        if general:
            m["gamma"] = ln_gamma
        in_maps.append(m)

    global _last_in_maps
    _last_in_maps = in_maps

    res = run_bass_kernel_spmd(nc, in_maps, list(range(NCORES)))

    out = np.empty((B, N, DOUT), dtype=np.float32)
    for k in range(NCORES):
        bk, hk = k // 2, k % 2
        out[bk, hk * half : (hk + 1) * half] = res.results[k]["out"]
    return out


# revision 12
# speedup vs baseline: 1.2441x; 1.2441x over previous
"""CausalLocalSGU Trainium2 kernel.

Reference computation (per batch b):
  split x[b] channels -> res (first 1024), gate_in (last 1024)
  per 128-token window block j: z_j = LayerNorm(gate_in_j) * gamma + beta
  gate_out_j[m, c] = sum_n W[h(c), m, n] * [z_{j-1}; z_j][n, c] + bias[h(c), m]
      (W masked causally: keep [m, n] where n <= m + 128; z_{-1} = 0)
  out_j = gate_out_j * res_j

Sharding: 8 cores; core k handles batch k//2, token half k%2 (2048 tokens =
16 window blocks) plus a one-block halo on the left (zeros for even cores).
The LN of the halo block is recomputed locally -> no collectives.

Device pipeline per block (fast path: gamma==1, beta==0):
  LN stats via two ScalarE passes with accum_out (sum, sum of squares), tiny
  VectorE ops for mu/var, rstd via ACT sqrt + DVE reciprocal, normalize on
  ScalarE (Identity, scale=rstd, bias=-mu*rstd) into a bf16 z tile, 8 bf16
  matmuls (prev/current window per head; causal mask + transpose pre-applied
  on host), then one scalar_tensor_tensor per head on VectorE:
  out = (psum + bias[h]) * res, and DMA out.

DMA is batched 4 window blocks per transfer (1-2 MiB per dma_start).

The gate half is cast to bf16 on the host: the einsum term it feeds
contributes ~7e-5 of the output magnitude (weights ~1e-5), so bf16 there
perturbs the output by ~3e-7 relative while halving LN traffic and PE time.

General path (gamma != 1 or beta != 0) additionally multiplies z by gamma
(DVE) and folds beta+bias through an fp32 K=2 matmul:
  W @ (gamma*z + beta) = W @ (gamma*z) + S[m]*beta[c],  S[m] = sum_n W[m, n]
(S excludes the first window's columns when the previous block is zero pad).
"""

import ml_dtypes
import numpy as np

import concourse.bacc as bacc
import concourse.bass as bass
import concourse.tile as tile
from concourse import mybir
from concourse.bass_utils import run_bass_kernel_spmd

F32 = mybir.dt.float32
BF16 = mybir.dt.bfloat16

HEADS = 4
W = 128            # window
DIM = 2048
DOUT = 1024        # dim // 2
DHEAD = DOUT // HEADS  # 256
B = 4
N = 4096
NCORES = 8
BLK_PER_CORE = (N // 2) // W   # 16
MACRO = 4          # window blocks per DMA batch
LN_EPS = 1e-5

# fp32 consts layout (columns of the [128, 2056] consts tensor)
_BIAS0 = 0          # 4 cols: bias[h, :] as per-partition columns
_EXF0 = 8           # [2, 512] rows 0..1: bias / S for the first block
_EXR0 = 520         # [2, 512] rows 0..1: bias / S for the other blocks
_RHSX0 = 1032       # [2, 1024] row 0: ones, row 1: beta
_CONSTS_COLS = 2056

_NC_CACHE: dict = {}
_last_in_maps: list = []


def _build_nc(general: bool) -> bass.Bass:
    nc = bacc.Bacc(
        trn_type="TRN2",
        target_bir_lowering=False,
        debug=False,
        num_devices=NCORES,
    )
    nblk = BLK_PER_CORE  # output blocks per core; +1 halo block for gate
    res_sh = nc.dram_tensor("res_sh", [nblk * W, DOUT], F32, kind="ExternalInput").ap()
    gate_sh = nc.dram_tensor(
        "gate_sh", [(nblk + 1) * W, DOUT], BF16, kind="ExternalInput"
    ).ap()
    consts = nc.dram_tensor("consts", [W, _CONSTS_COLS], F32, kind="ExternalInput").ap()
    consts_bf = nc.dram_tensor(
        "consts_bf", [W, 2 * HEADS * W], BF16, kind="ExternalInput"
    ).ap()
    if general:
        gamma = nc.dram_tensor("gamma", [DOUT], F32, kind="ExternalInput").ap()
    out = nc.dram_tensor("out", [nblk * W, DOUT], F32, kind="ExternalOutput").ap()

    ident = mybir.ActivationFunctionType.Identity
    alu = mybir.AluOpType

    with tile.TileContext(nc) as tc:
        with (
            tc.tile_pool(name="singles", bufs=1) as singles,
            tc.tile_pool(name="gpool", bufs=3) as gpool,
            tc.tile_pool(name="rpool", bufs=3) as rpool,
            tc.tile_pool(name="opool", bufs=3) as opool,
            tc.tile_pool(name="zpool", bufs=4) as zpool,
            tc.tile_pool(name="spool", bufs=6) as spool,
            tc.tile_pool(name="scratch", bufs=2) as scratch,
            tc.tile_pool(name="ppool", bufs=4, space="PSUM") as ppool,
        ):
            consts_t = singles.tile([W, _CONSTS_COLS], F32)
            nc.sync.dma_start(out=consts_t, in_=consts)
            wt_t = singles.tile([W, 2 * HEADS * W], BF16)
            nc.sync.dma_start(out=wt_t, in_=consts_bf)
            exf_t = consts_t[0:2, _EXF0 : _EXF0 + HEADS * W]
            exr_t = consts_t[0:2, _EXR0 : _EXR0 + HEADS * W]
            rhsx_t = consts_t[0:2, _RHSX0 : _RHSX0 + DOUT]

            eps_t = singles.tile([128, 1], F32)
            nc.vector.memset(eps_t, LN_EPS)
            if general:
                gamma_t = singles.tile([128, DOUT], F32)
                nc.gpsimd.dma_start(
                    out=gamma_t,
                    in_=bass.AP(
                        tensor=gamma.tensor,
                        offset=gamma.offset,
                        ap=[[0, 128]] + list(gamma.ap),
                    ),
                )

            def layer_norm(gate):
                """gate: [W, DOUT] bf16 SBUF slice -> z (bf16 tile)."""
                stats = spool.tile([W, 2, 6], F32, tag="stats")
                nc.vector.bn_stats(out=stats[:, 0], in_=gate[:, :512])
                nc.vector.bn_stats(out=stats[:, 1], in_=gate[:, 512:])
                mv = spool.tile([W, 2], F32, tag="mv")
                nc.vector.bn_aggr(out=mv, in_=stats)
                sd = spool.tile([W, 1], F32, tag="sd")
                nc.scalar.activation(
                    out=sd,
                    in_=mv[:, 1:2],
                    func=mybir.ActivationFunctionType.Sqrt,
                    bias=eps_t,
                )
                rstd = spool.tile([W, 1], F32, tag="rstd")
                nc.vector.reciprocal(out=rstd, in_=sd)
                negmu = spool.tile([W, 1], F32, tag="negmu")
                nc.vector.tensor_scalar(
                    out=negmu,
                    in0=mv[:, 0:1],
                    scalar1=rstd,
                    scalar2=-1.0,
                    op0=alu.mult,
                    op1=alu.mult,
                )
                z = zpool.tile([W, DOUT], BF16, tag="z")
                nc.scalar.activation(
                    out=z, in_=gate, func=ident, bias=negmu, scale=rstd
                )
                if general:
                    nc.vector.tensor_mul(z, z, gamma_t)
                return z

            # halo block
            gate0 = gpool.tile([W, DOUT], BF16, tag="gate0")
            nc.sync.dma_start(out=gate0, in_=gate_sh[0:W, :])
            z_prev = layer_norm(gate0)

            nmac = nblk // MACRO
            for m in range(nmac):
                g4 = gpool.tile([W, MACRO, DOUT], BF16, tag="g4")
                nc.sync.dma_start(
                    out=g4,
                    in_=gate_sh[(1 + m * MACRO) * W : (1 + (m + 1) * MACRO) * W, :]
                    .rearrange("(b p) d -> p b d", p=W),
                )
                r4 = rpool.tile([W, MACRO, DOUT], F32, tag="r4")
                nc.sync.dma_start(
                    out=r4,
                    in_=res_sh[m * MACRO * W : (m + 1) * MACRO * W, :]
                    .rearrange("(b p) d -> p b d", p=W),
                )
                o4 = opool.tile([W, MACRO, DOUT], F32, tag="o4")
                for s in range(MACRO):
                    blk = m * MACRO + s       # output block index 0..15
                    z = layer_norm(g4[:, s, :])
                    psum = ppool.tile([W, DOUT], F32, tag="psum")
                    ex_t = exf_t if blk == 0 else exr_t
                    for h in range(HEADS):
                        ps = psum[:, h * DHEAD : (h + 1) * DHEAD]
                        zp = z_prev[:, h * DHEAD : (h + 1) * DHEAD]
                        zc = z[:, h * DHEAD : (h + 1) * DHEAD]
                        if general:
                            nc.tensor.matmul(
                                ps,
                                ex_t[:, h * W : (h + 1) * W],
                                rhsx_t[:, h * DHEAD : (h + 1) * DHEAD],
                                start=True,
                                stop=False,
                            )
                        nc.tensor.matmul(
                            ps,
                            wt_t[:, (2 * h) * W : (2 * h + 1) * W],
                            zp,
                            start=not general,
                            stop=False,
                        )
                        nc.tensor.matmul(
                            ps,
                            wt_t[:, (2 * h + 1) * W : (2 * h + 2) * W],
                            zc,
                            start=False,
                            stop=True,
                        )
                    if general:
                        # bias/beta already folded in via the extras matmul
                        nc.vector.tensor_mul(o4[:, s, :], psum, r4[:, s, :])
                    else:
                        gb = scratch.tile([W, DOUT], F32, tag="gb")
                        for h in range(HEADS):
                            hs = slice(h * DHEAD, (h + 1) * DHEAD)
                            nc.scalar.activation(
                                out=gb[:, hs],
                                in_=psum[:, hs],
                                func=ident,
                                bias=consts_t[:, _BIAS0 + h : _BIAS0 + h + 1],
                                scale=1.0,
                            )
                        nc.vector.tensor_mul(o4[:, s, :], gb, r4[:, s, :])
                    z_prev = z
                nc.gpsimd.dma_start(
                    out=out[m * MACRO * W : (m + 1) * MACRO * W, :]
                    .rearrange("(b p) d -> p b d", p=W),
                    in_=o4,
                )
    if not nc.is_finalized():
        nc.finalize()
    return nc


def _host_prep(weight, bias, ln_beta):
    j = np.arange(2 * W)[None, :]
    i_ = np.arange(W)[:, None]
    mask = (j <= i_ + W).astype(np.float32)          # [W, 2W]
    wm = weight * mask[None]                         # [H, W, 2W]
    wT = np.zeros((W, 2 * HEADS, W), dtype=np.float32)
    for h in range(HEADS):
        wT[:, 2 * h] = wm[h, :, :W].T                # A_h: prev-window cols
        wT[:, 2 * h + 1] = wm[h, :, W:].T            # B_h: current-window cols
    wT = wT.reshape(W, 2 * HEADS * W)

    s_full = wm.sum(-1).reshape(HEADS * W)
    s_first = wm[:, :, W:].sum(-1).reshape(HEADS * W)
    bias_flat = bias.reshape(HEADS * W)

    def consts_for(first_has_prev: bool):
        c = np.zeros((W, _CONSTS_COLS), dtype=np.float32)
        for h in range(HEADS):
            c[:, _BIAS0 + h] = bias[h]
        c[0, _EXF0 : _EXF0 + HEADS * W] = bias_flat
        c[1, _EXF0 : _EXF0 + HEADS * W] = s_full if first_has_prev else s_first
        c[0, _EXR0 : _EXR0 + HEADS * W] = bias_flat
        c[1, _EXR0 : _EXR0 + HEADS * W] = s_full
        c[0, _RHSX0 : _RHSX0 + DOUT] = 1.0
        c[1, _RHSX0 : _RHSX0 + DOUT] = ln_beta
        return c

    consts_bf = np.ascontiguousarray(wT.astype(ml_dtypes.bfloat16))
    return consts_for(False), consts_for(True), consts_bf


def kernel(x, weight, bias, ln_gamma, ln_beta):
    x = np.ascontiguousarray(x, dtype=np.float32)
    weight = np.asarray(weight, dtype=np.float32)
    bias = np.asarray(bias, dtype=np.float32)
    ln_gamma = np.asarray(ln_gamma, dtype=np.float32)
    ln_beta = np.asarray(ln_beta, dtype=np.float32)

    consts_even, consts_odd, consts_bf = _host_prep(weight, bias, ln_beta)

    general = not (np.all(ln_gamma == 1.0) and np.all(ln_beta == 0.0))
    if general not in _NC_CACHE:
        _NC_CACHE[general] = _build_nc(general)
    nc = _NC_CACHE[general]

    half = N // 2
    gate_bf = np.ascontiguousarray(x[:, :, DOUT:]).astype(ml_dtypes.bfloat16)
    in_maps = []
    for k in range(NCORES):
        bk, hk = k // 2, k % 2
        res_sh = np.ascontiguousarray(x[bk, hk * half : (hk + 1) * half, :DOUT])
        if hk == 0:
            halo = np.zeros((W, DOUT), dtype=ml_dtypes.bfloat16)
        else:
            halo = gate_bf[bk, half - W : half]
        gate_sh = np.ascontiguousarray(
            np.concatenate([halo, gate_bf[bk, hk * half : (hk + 1) * half]], axis=0)
        )
        m = {
            "res_sh": res_sh,
            "gate_sh": gate_sh,
            "consts": consts_odd if hk == 1 else consts_even,
            "consts_bf": consts_bf,
        }
        if general:
            m["gamma"] = ln_gamma
        in_maps.append(m)

    global _last_in_maps
    _last_in_maps = in_maps

    res = run_bass_kernel_spmd(nc, in_maps, list(range(NCORES)))

    out = np.empty((B, N, DOUT), dtype=np.float32)
    for k in range(NCORES):
        bk, hk = k // 2, k % 2
        out[bk, hk * half : (hk + 1) * half] = res.results[k]["out"]
    return out
